# revision 8
# baseline (speedup 1.0000x reference)
"""Trainium2 Bass kernel for EfmLSTM (signature-gated LSTM), 8-core data-parallel.

Strategy
--------
Data-parallel over batch: B=64 -> 8 cores x B_loc=8. Everything on-chip uses a
"units-on-partition" transposed layout so the sequential scan needs no
transposes at all:

  h^T, c^T, f^T, gate tensors are [128 partitions, u*8+b] where unit = 128*u+p.

Per timestep (per core):
  gates^T [128, 96] (12 chunks of (gate_type, unit_chunk) x 8 batch) =
    sum_k W_rec[k-chunk, m-chunk]^T-stationary  @  h^T[:, k-chunk]  (48 matmuls,
    bf16, N=8 moving)  accumulated in one PSUM bank, + x^T_t (precomputed) via DVE,
  then ACT sigmoid/tanh on [128, 32] slices, DVE for the c/h updates.

x^T = inputs @ W_in and f^T = sigmoid(signatures @ W_f + b_f) are precomputed
per 128-step chunk with full-width matmuls (cheap), with biases folded in at
PSUM->SBUF eviction time.

Host side pre-permutes weights into gate order [c~, i, o] and pre-transposes /
pre-casts inputs to bf16, so the device never transposes anything.
"""

import numpy as np
import ml_dtypes

# Problem shapes (hardcoded per harness contract)
B, T, F = 64, 1024, 256
U = 512
SIG = 31
NCORES = 8
BL = B // NCORES  # 8 batch per core

T_CHUNK = 128
N_CHUNKS = T // T_CHUNK
KC = U // 128   # 4 k-chunks of h/units
MC = (3 * U) // 128  # 12 gate-column chunks
FC = F // 128   # 2 k-chunks of input features

_cache = {}


def _split_excess_waits(nc, limit=1):
    """This walrus build rejects >1 sync-wait command per instruction
    ('Too many sync wait commands', CoreV2/V3 setupSyncWait). Hoist excess
    waits onto same-engine NoOp instructions inserted just before the
    offending instruction — the engine sequencer processes its queue in
    order, so the waits still complete before the instruction issues."""
    import concourse.mybir as mybir
    import bass_rust as _br

    cnt = 0
    for f in nc.m.functions:
        for b in f.blocks:
            il = b.instructions
            if not any(
                i.sync_info and i.sync_info.on_wait and len(i.sync_info.on_wait) > limit
                for i in il
            ):
                continue
            new = []
            for inst in il:
                si = inst.sync_info
                waits = list(si.on_wait) if si and si.on_wait else []
                if len(waits) > limit:
                    for w in waits[:-limit]:
                        nop = mybir.InstNoOp(name=f"wsplit_{cnt}", ins=[], outs=[])
                        cnt += 1
                        nop.engine = inst.engine
                        nop.sync_info = _br.SyncInfo(on_wait=[w], on_update=[])
                        new.append(nop)
                    si.on_wait = waits[-limit:]
                new.append(inst)
            il[:] = new
    return cnt


def _build_nc(compute_dt_name="bfloat16"):
    import concourse.bass as bass
    import concourse.mybir as mybir
    import concourse.tile as tile
    from contextlib import ExitStack


    fp32 = mybir.dt.float32
    bf16 = mybir.dt.bfloat16
    cdt = getattr(mybir.dt, compute_dt_name)
    AF = mybir.ActivationFunctionType
    ALU = mybir.AluOpType

    nc = bass.Bass()

    # --- DRAM parameters (per-core shards; host pre-transposed/pre-cast) ---
    # inputs_t: [F, BL, T] bf16 ; sig_t: [SIG, BL, T] bf16
    x_in = nc.declare_dram_parameter("inputs_t", [F, BL, T], cdt, isOutput=False)
    sig_in = nc.declare_dram_parameter("sig_t", [SIG, BL, T], cdt, isOutput=False)
    # weights, host-permuted to gate order [c~, i, o]:
    # wrec: [U, 3U] -> rearranged host-side to [128, KC*MC*128] (k-major blocks)
    wrec_in = nc.declare_dram_parameter("wrec", [128, KC * MC * 128], cdt, isOutput=False)
    win_in = nc.declare_dram_parameter("win", [128, FC * MC * 128], cdt, isOutput=False)
    wsig_in = nc.declare_dram_parameter("wsig", [SIG, U], cdt, isOutput=False)
    bias_g_in = nc.declare_dram_parameter("bias_g", [128, MC], fp32, isOutput=False)
    bias_f_in = nc.declare_dram_parameter("bias_f", [128, KC], fp32, isOutput=False)
    h_out = nc.declare_dram_parameter("h_out", [128, KC * BL], fp32, isOutput=True)

    with ExitStack() as ctx:
        tc = ctx.enter_context(tile.TileContext(nc))

        const = ctx.enter_context(tc.tile_pool(name="const", bufs=1))
        state = ctx.enter_context(tc.tile_pool(name="state", bufs=1))
        xpool = ctx.enter_context(tc.tile_pool(name="xpool", bufs=2))
        fpool = ctx.enter_context(tc.tile_pool(name="fpool", bufs=2))
        inpool = ctx.enter_context(tc.tile_pool(name="inpool", bufs=2))
        sigpool = ctx.enter_context(tc.tile_pool(name="sigpool", bufs=2))
        work = ctx.enter_context(tc.tile_pool(name="work", bufs=2))
        psum_g = ctx.enter_context(tc.tile_pool(name="psum_g", bufs=2, space="PSUM"))
        psum_p = ctx.enter_context(tc.tile_pool(name="psum_p", bufs=4, space="PSUM"))

        # --- load weights/biases into SBUF once ---
        wrec = const.tile([128, KC * MC * 128], cdt)
        nc.sync.dma_start(out=wrec[:], in_=wrec_in[:])
        win = const.tile([128, FC * MC * 128], cdt)
        nc.sync.dma_start(out=win[:], in_=win_in[:])
        wsig = const.tile([SIG, U], cdt)
        nc.sync.dma_start(out=wsig[:], in_=wsig_in[:])
        bias_g = const.tile([128, MC], fp32)
        nc.sync.dma_start(out=bias_g[:], in_=bias_g_in[:])
        bias_f = const.tile([128, KC], fp32)
        nc.sync.dma_start(out=bias_f[:], in_=bias_f_in[:])

        # --- persistent state (units-on-partition) ---
        h_bf = state.tile([128, KC * BL], cdt)      # h^T bf16, col = 8*k + b
        c_st = state.tile([128, KC * BL], fp32)     # c^T fp32
        nc.any.memset(h_bf[:], 0.0)
        nc.any.memset(c_st[:], 0.0)

        # x chunk: col = t*96 + j*8 + b (j = gate chunk 0..11, order [c~,i,o])
        # f chunk: col = t*32 + u*8 + b
        def precompute_chunk(ci):
            t0 = ci * T_CHUNK
            in_sb = inpool.tile([128, FC * BL * T_CHUNK], cdt, tag="in_sb")
            # dram [F=(FC p), BL, T] slice -> sbuf [p, (k b t)]; one DMA per
            # k-block keeps each AP at <=3 dims (DMA balancing limit)
            srcv = x_in.rearrange("(k p) b t -> p k b t", p=128)
            in_sbv = in_sb.rearrange("p (k b t) -> p k b t", k=FC, b=BL)
            for k in range(FC):
                nc.sync.dma_start(out=in_sbv[:, k, :, :],
                                  in_=srcv[:, k, :, t0:t0 + T_CHUNK])
            sig_sb = sigpool.tile([SIG, BL * T_CHUNK], cdt, tag="sig_sb")
            nc.sync.dma_start(out=sig_sb.rearrange("p (b t) -> p b t", b=BL),
                              in_=sig_in[:, :, t0:t0 + T_CHUNK])

            x_sb = xpool.tile([128, T_CHUNK * MC * BL], cdt, tag="x_sb")
            f_sb = fpool.tile([128, T_CHUNK * KC * BL], cdt, tag="f_sb")

            NT = 512 // BL  # timesteps covered per 512-wide matmul
            in_sb4 = in_sb.rearrange("p (k b t) -> p k b t", k=FC, b=BL)
            # (b, t)-ordered views matching the psum (b-major, t-minor) layout
            x_sb4 = x_sb.rearrange("p (t m b) -> p m b t", m=MC, b=BL)
            f_sb4 = f_sb.rearrange("p (t u b) -> p u b t", u=KC, b=BL)
            sig_sb3 = sig_sb.rearrange("p (b t) -> p b t", b=BL)
            for j in range(MC):
                for th in range(T_CHUNK // NT):
                    ps = psum_p.tile([128, 512], fp32, tag="ps_pre")
                    for k in range(FC):
                        nc.tensor.matmul(
                            ps[:],
                            lhsT=win[:, (k * MC + j) * 128:(k * MC + j + 1) * 128],
                            rhs=in_sb4[:, k, :, th * NT:(th + 1) * NT],
                            start=(k == 0), stop=(k == FC - 1),
                        )
                    # evict with bias; out col = t*96 + j*8 + b ; psum col = b*NT + t
                    dst = x_sb4[:, j, :, th * NT:(th + 1) * NT]
                    nc.scalar.activation(
                        dst, ps[:], AF.Identity, bias=bias_g[:, j:j + 1])
            for u in range(KC):
                for th in range(T_CHUNK // NT):
                    ps = psum_p.tile([128, 512], fp32, tag="ps_pre")
                    nc.tensor.matmul(
                        ps[:],
                        lhsT=wsig[:, u * 128:(u + 1) * 128],
                        rhs=sig_sb3[:, :, th * NT:(th + 1) * NT],
                        start=True, stop=True,
                    )
                    dst = f_sb4[:, u, :, th * NT:(th + 1) * NT]
                    nc.scalar.activation(
                        dst, ps[:], AF.Sigmoid, bias=bias_f[:, u:u + 1])
            return x_sb, f_sb

        for ci in range(N_CHUNKS):
            x_sb, f_sb = precompute_chunk(ci)
            for tt in range(T_CHUNK):
                t = ci * T_CHUNK + tt
                last = (t == T - 1)
                pg = psum_g.tile([128, MC * BL], fp32, tag="pg")
                # 48 matmuls: m-outer (c~ 0-3, i 4-7, o 8-11), k-inner
                for j in range(MC):
                    for k in range(KC):
                        nc.tensor.matmul(
                            pg[:, j * BL:(j + 1) * BL],
                            lhsT=wrec[:, (k * MC + j) * 128:(k * MC + j + 1) * 128],
                            rhs=h_bf[:, k * BL:(k + 1) * BL],
                            start=(k == 0), stop=(k == KC - 1),
                        )
                g = work.tile([128, MC * BL], fp32, tag="g")
                # g = pg + x_t
                nc.vector.scalar_tensor_tensor(
                    g[:], pg[:], 1.0, x_sb[:, tt * MC * BL:(tt + 1) * MC * BL],
                    op0=ALU.mult, op1=ALU.add)
                s = work.tile([128, MC * BL], fp32, tag="s")
                W = KC * BL  # 32
                nc.scalar.activation(s[:, 0:W], g[:, 0:W], AF.Tanh)          # c~
                nc.scalar.activation(s[:, W:2 * W], g[:, W:2 * W], AF.Sigmoid)  # i
                nc.scalar.activation(s[:, 2 * W:3 * W], g[:, 2 * W:3 * W], AF.Sigmoid)  # o
                tmp = work.tile([128, W], fp32, tag="tmp")
                # tmp = i * c~
                nc.vector.scalar_tensor_tensor(
                    tmp[:], s[:, W:2 * W], 1.0, s[:, 0:W], op0=ALU.mult, op1=ALU.mult)
                # c = f*c
                nc.vector.scalar_tensor_tensor(
                    c_st[:], f_sb[:, tt * W:(tt + 1) * W], 1.0, c_st[:],
                    op0=ALU.mult, op1=ALU.mult)
                # c += tmp
                nc.vector.scalar_tensor_tensor(
                    c_st[:], c_st[:], 1.0, tmp[:], op0=ALU.mult, op1=ALU.add)
                tc_t = work.tile([128, W], fp32, tag="tc")
                nc.scalar.activation(tc_t[:], c_st[:], AF.Tanh)
                if not last:
                    nc.vector.scalar_tensor_tensor(
                        h_bf[:], s[:, 2 * W:3 * W], 1.0, tc_t[:],
                        op0=ALU.mult, op1=ALU.mult)
                else:
                    h_f = state.tile([128, KC * BL], fp32)
                    nc.vector.scalar_tensor_tensor(
                        h_f[:], s[:, 2 * W:3 * W], 1.0, tc_t[:],
                        op0=ALU.mult, op1=ALU.mult)
                    nc.sync.dma_start(out=h_out[:], in_=h_f[:])

    _split_excess_waits(nc)
    return nc


def _prep_host_inputs(inputs, signatures, forget_kernel, input_kernel,
                      recurrent_kernel, bias, cdt=ml_dtypes.bfloat16):
    """Host-side shard + permute + transpose + cast. Returns in_maps list."""
    # gate order in reference: [i, c~, o]; ours: [c~, i, o]
    perm = np.concatenate([np.arange(U, 2 * U), np.arange(0, U), np.arange(2 * U, 3 * U)])
    win_p = input_kernel[:, perm]          # [F, 3U]
    wrec_p = recurrent_kernel[:, perm]     # [U, 3U]
    b_i, b_f, b_c, b_o = np.split(bias, 4)
    bias_g = np.concatenate([b_c, b_i, b_o])  # per permuted gate col, [3U]

    # wrec blocks: [128, (k*MC + j)*128 + c] = wrec_p[128*k + p, 128*j + c]
    wr = wrec_p.reshape(KC, 128, MC, 128).transpose(1, 0, 2, 3).reshape(128, KC * MC * 128)
    wi = win_p.reshape(FC, 128, MC, 128).transpose(1, 0, 2, 3).reshape(128, FC * MC * 128)
    bg = bias_g.reshape(MC, 128).T.copy()          # [128, MC]
    bf_ = b_f.reshape(KC, 128).T.copy()            # [128, KC]

    wr = wr.astype(cdt)
    wi = wi.astype(cdt)
    wsig = forget_kernel.astype(cdt)               # [SIG, U]

    in_maps = []
    for c in range(NCORES):
        bsl = slice(c * BL, (c + 1) * BL)
        # [BL, T, F] -> [F, BL, T]
        x_t = np.ascontiguousarray(inputs[bsl].transpose(2, 0, 1)).astype(cdt)
        s_t = np.ascontiguousarray(signatures[bsl].transpose(2, 0, 1)).astype(cdt)
        in_maps.append({
            "inputs_t": x_t, "sig_t": s_t, "wrec": wr, "win": wi,
            "wsig": wsig, "bias_g": bg.astype(np.float32),
            "bias_f": bf_.astype(np.float32),
        })
    return in_maps


def kernel(inputs, signatures, forget_kernel, input_kernel, recurrent_kernel,
           bias, _trace=False):
    inputs = np.asarray(inputs, dtype=np.float32)
    signatures = np.asarray(signatures, dtype=np.float32)
    forget_kernel = np.asarray(forget_kernel, dtype=np.float32)
    input_kernel = np.asarray(input_kernel, dtype=np.float32)
    recurrent_kernel = np.asarray(recurrent_kernel, dtype=np.float32)
    bias = np.asarray(bias, dtype=np.float32)

    from concourse.bass_utils import run_bass_kernel_spmd

    if "nc" not in _cache:
        _cache["nc"] = _build_nc()
    nc = _cache["nc"]

    in_maps = _prep_host_inputs(inputs, signatures, forget_kernel,
                                input_kernel, recurrent_kernel, bias)
    res = run_bass_kernel_spmd(nc, in_maps, list(range(NCORES)), trace=_trace)

    out = np.empty((B, U), np.float32)
    for c in range(NCORES):
        hT = res.results[c]["h_out"]                  # [128, KC*BL]
        h = hT.reshape(128, KC, BL).transpose(2, 1, 0).reshape(BL, U)
        out[c * BL:(c + 1) * BL] = h
    if _trace:
        return out, res
    return out
